# revision 66
# baseline (speedup 1.0000x reference)
"""Causal self-attention Trainium2 kernel (B=4, S=2048, D=1024, H=16, Dh=64).

Sharding: 8 cores = 4 batches x 2 head-groups (8 heads each). Each core
computes the qkv projection for its heads, causal attention, and a partial
output projection; the host sums the two partials per batch and adds b_out.

Design (all choices A/B-measured on hardware):
  - fp16 storage everywhere (x, W, qT/kT, v, eT, outT, DRAM out); PSUM
    stays fp32. fp16 streams ~2-5x faster per matmul than fp32r/bf16 on
    this toolchain and has no N>=256 restriction, so diagonal blocks use
    exact widths (512/384/256/128).
  - weights loaded once, SBUF-resident; one-time loads ride the gpsimd
    SWDGE queue so x loads on the sync queue are never blocked.
  - fused pipeline, emission order = scheduler priority: after each
    attention pair-stream emit its softmax-normalize (C1), the next
    chunk's projection piece, and the PREVIOUS chunk's out-projection
    tile, so the PE always has fill work during exp latency. At the
    chunk head, pair 0's q/k plus all v-tiles go first so exp starts
    early; remaining pairs' q/k slide into the exp gaps.
  - QK per head = K=64 matmul into its own PSUM bank (auto row-tiles
    (0,0)/(64,0); same-bank row-tile pairs RACE - never do that).
  - exp as two per-head ACT instructions into per-head eT tiles (finer
    chain granularity lets PV(h0) start under exp(h1)).
  - causal mask = post-exp DVE multiply by a 0/1 triangle (keeps the
    mask off the LDW-serialized PE).
  - softmax denominators via the ones-column-in-v trick: PV emits den at
    a known partition; 1/den via reciprocal_approx_fast, broadcast to
    128 partitions by a tiny selector matmul, applied with the bv bias
    fold (attn sums to 1, so out = pv/den + bv needs no v pre-bias).
"""
import os
import numpy as np
from collections import deque
from contextlib import ExitStack

GPSIMD_DMA = os.environ.get("KV2_GPSIMD_DMA", "1") == "1"
EXP2 = os.environ.get("KV2_EXP2", "1") == "1"
SCHED = os.environ.get("KV2_SCHED", "flat")  # flat | legacy
FILL_FUDGE = float(os.environ.get("KV2_FUDGE", "1.15"))
C1_CREDIT = float(os.environ.get("KV2_C1_CREDIT", "1500"))
DEBT_CLAMP = float(os.environ.get("KV2_CLAMP", "2000"))
RESERVE = int(os.environ.get("KV2_RESERVE", "3"))
A_COST = FILL_FUDGE * 8 * 512 / 2.4      # one q/k/v projection unit
C2_COST = FILL_FUDGE * 4 * 512 / 2.4     # one out-proj half-piece
EXP_SPAN = os.environ.get("KV2_EXP_SPAN", "0") == "1"
PHASES = os.environ.get("KV2_PHASES", "abc")
B_PARTS = os.environ.get("KV2_B_PARTS", "qmep")  # q=QK m=mask e=exp p=PV
SKIP_A = os.environ.get("KV2_SKIP_A", "0") == "1"
# "direct" (skip the rounding copy) passes CoreSim but the neuronxcc BIR
# verifier rejects a non-rounded fp32r matmult operand — keep the copy.
_AR = os.environ.get("KV2_APPROX_RECIP", "1")
APPROX_RECIP = _AR if _AR == "direct" else _AR == "1"
# ACT Reciprocal is hard-blocked by bass (accuracy); keep the DVE recip.
ACT_RECIP = os.environ.get("KV2_ACT_RECIP", "0") == "1"
N_WARM = int(os.environ.get("KV2_WARM", "8"))
MASK_DVE = os.environ.get("KV2_MASK_DVE", "1") == "1"

import concourse.bass as bass
import concourse.tile as tile
from concourse import bacc, mybir
from concourse.bass_utils import run_bass_kernel_spmd

P = 128
D_MODEL = 1024
NHEAD = 16
HEAD_DIM = 64
B = 4
S = 2048
N_CORES = 8
HEADS_LOC = 8           # heads per core
NPAIR = HEADS_LOC // 2  # head pairs per core
DL = HEADS_LOC * HEAD_DIM  # local dims = 512
NDT = D_MODEL // P      # 8 d-tiles
NQC = S // 512          # sq chunks of 512
NST = S // P            # s tiles of 128
VSLOT = 192             # per-pair v slot: v_h0(64) | ones(64) | v_h1(64)
F32 = mybir.dt.float32
F32R = mybir.dt.float32r
F16 = mybir.dt.float16
SCALE = HEAD_DIM ** -0.5

_NC_CACHE = {}


def build_nc(loop=None, loop_n=17):
    key = (loop, loop_n if loop else 0)
    if key in _NC_CACHE:
        return _NC_CACHE[key]
    nc = bacc.Bacc("TRN2", target_bir_lowering=False, debug=False,
                   num_devices=N_CORES)
    xT = nc.dram_tensor("xT", [D_MODEL, S], F16, kind="ExternalInput").ap()
    wq = nc.dram_tensor("wq", [D_MODEL, DL], F16, kind="ExternalInput").ap()
    wk = nc.dram_tensor("wk", [D_MODEL, DL], F16, kind="ExternalInput").ap()
    wv = nc.dram_tensor("wv", [D_MODEL, DL], F16, kind="ExternalInput").ap()
    wout = nc.dram_tensor("wout", [DL, D_MODEL], F16, kind="ExternalInput").ap()
    bq = nc.dram_tensor("bq", [P, NPAIR], F32, kind="ExternalInput").ap()
    bk = nc.dram_tensor("bk", [P, NPAIR], F32, kind="ExternalInput").ap()
    bv = nc.dram_tensor("bv", [P, NPAIR], F32, kind="ExternalInput").ap()
    ident = nc.dram_tensor("ident", [P, P], F16, kind="ExternalInput").ap()
    negmask = nc.dram_tensor("negmask", [P, P], F16, kind="ExternalInput").ap()
    binmask = nc.dram_tensor("binmask", [P, P], F16, kind="ExternalInput").ap()
    sel = nc.dram_tensor("sel", [65, P], F32R, kind="ExternalInput").ap()
    onesv = nc.dram_tensor("onesv", [P, 4, NPAIR, 64], F16,
                           kind="ExternalInput").ap()
    out = nc.dram_tensor("out", [S, D_MODEL], F16, kind="ExternalOutput").ap()

    wqr = wq.rearrange("(a p) n -> p a n", p=P)
    wkr = wk.rearrange("(a p) n -> p a n", p=P)
    wvr = wv.rearrange("(a p) n -> p a n", p=P)
    woutr = wout.rearrange("(a p) (b n) -> p a b n", p=P, n=512)
    xTr = xT.rearrange("(a p) s -> p a s", p=P)

    with tile.TileContext(nc) as tc, ExitStack() as ctx, \
         nc.allow_low_precision(reason="fp16 storage is within tolerance"):
        # ---- persistent SBUF ----
        persist = ctx.enter_context(tc.tile_pool(name="persist", bufs=1))
        qTt = [[persist.tile([P, 512], F16, tag=f"qT{p}_{qc}",
                             name=f"qT{p}_{qc}")
                for qc in range(NQC)] for p in range(NPAIR)]
        kTt = [[persist.tile([P, 512], F16, tag=f"kT{p}_{qc}",
                             name=f"kT{p}_{qc}")
                for qc in range(NQC)] for p in range(NPAIR)]
        vaugt = [persist.tile([P, 4, NPAIR, VSLOT], F16, tag=f"v{qc}",
                              name=f"vaug{qc}")
                 for qc in range(NQC)]
        outT = persist.tile([P, NPAIR, S], F16, tag="outT", name="outT")
        rct = [persist.tile([65, NQC, 512], F32R, tag=f"rc{pp}",
                            name=f"rc{pp}") for pp in range(NPAIR)]

        consts = ctx.enter_context(tc.tile_pool(name="consts", bufs=1))
        wq_sb = consts.tile([P, NDT, DL], F16, tag="wq")
        wk_sb = consts.tile([P, NDT, DL], F16, tag="wk")
        wv_sb = consts.tile([P, NDT, DL], F16, tag="wv")
        wout_sb = consts.tile([P, NPAIR, 2, 512], F16, tag="wout")
        bq_sb = consts.tile([P, NPAIR], F32, tag="bq")
        bk_sb = consts.tile([P, NPAIR], F32, tag="bk")
        bv_sb = consts.tile([P, NPAIR], F32, tag="bv")
        id_sb = consts.tile([P, P], F16, tag="ident")
        nm_sb = consts.tile([P, P], F16, tag="negmask")
        bm_sb = consts.tile([P, P], F16, tag="binmask")
        sel_sb = consts.tile([65, P], F32R, tag="sel")
        # one-time loads on the gpsimd SWDGE queue, in PARALLEL with the x
        # chunk loads on the sync queue. Order = first-use order: pair-0
        # slices of wq/wk lead (first projection matmul ~1.6us in), then
        # wv + chunk-0 ones (PV(0) needs them ~7us in), then the rest.
        dmae = nc.gpsimd if GPSIMD_DMA else nc.sync
        # pair-0 wq/bq on the sync queue (starts fastest, ahead of the x
        # loads emitted by the body) so the first matmul isn't gated on
        # the gpsimd queue's startup latency.
        nc.sync.dma_start(wq_sb[:, :, 0:P], wqr[:, :, 0:P])
        nc.sync.dma_start(bq_sb, bq)
        dmae.dma_start(wk_sb[:, :, 0:P], wkr[:, :, 0:P])
        dmae.dma_start(bk_sb, bk)
        dmae.dma_start(wv_sb, wvr)
        dmae.dma_start(vaugt[0][:, :, :, 64:128], onesv)
        dmae.dma_start(bv_sb, bv)
        dmae.dma_start(wq_sb[:, :, P:], wqr[:, :, P:])
        dmae.dma_start(wk_sb[:, :, P:], wkr[:, :, P:])
        for qc in range(1, NQC):
            dmae.dma_start(vaugt[qc][:, :, :, 64:128], onesv)
        dmae.dma_start(id_sb, ident)
        dmae.dma_start(nm_sb, negmask)
        dmae.dma_start(bm_sb, binmask)
        dmae.dma_start(sel_sb, sel)
        dmae.dma_start(wout_sb, woutr)

        xts = {}

        def xload(qc, xtp):
            """x chunk qc as TWO tiles (dt 0..3 / 4..7) so consumers of the
            first half don't wait on the whole chunk's DMA. Chunk 0 rides
            the scalar/vector queues: in PARALLEL with wq on sync, so the
            first projection matmul starts ~2us earlier (the fixed
            DMA-to-semaphore latency makes serial queue order costly)."""
            cw = slice(qc * 512, (qc + 1) * 512)
            xa = xtp.tile([P, 4, 512], F16, tag="xt", name=f"x{qc}a")
            xb = xtp.tile([P, 4, 512], F16, tag="xt", name=f"x{qc}b")
            ea = nc.scalar if qc == 0 else nc.sync
            ea.dma_start(xa, xTr[:, 0:4, cw])
            ea.dma_start(xb, xTr[:, 4:8, cw])
            xts[qc] = (xa, xb)

        def xpart(qc, dt):
            xa, xb = xts[qc]
            return xa[:, dt] if dt < 4 else xb[:, dt - 4]

        def phase_a_piece(qc, p, xtp, psp):
            """1/4 of chunk qc's projection: qT/kT for pair p, v for st4=p."""
            if qc not in xts:
                xload(qc, xtp)
            psq = psp.tile([P, 512], F32, tag="ps", name=f"psq{p}")
            for dt in range(NDT):
                nc.tensor.matmul(psq, wq_sb[:, dt, bass.ts(p, P)],
                                 xpart(qc, dt),
                                 start=(dt == 0), stop=(dt == NDT - 1),
                                 skip_group_check=SKIP_A)
            nc.vector.tensor_scalar_add(qTt[p][qc], psq, bq_sb[:, p:p + 1])
            psk = psp.tile([P, 512], F32, tag="ps", name=f"psk{p}")
            for dt in range(NDT):
                nc.tensor.matmul(psk, wk_sb[:, dt, bass.ts(p, P)],
                                 xpart(qc, dt),
                                 start=(dt == 0), stop=(dt == NDT - 1),
                                 skip_group_check=SKIP_A)
            nc.vector.tensor_scalar_add(kTt[p][qc], psk, bk_sb[:, p:p + 1])

        def phase_a_vpiece(qc, st4, psp):
            """v-projection for seq-tile st4 of chunk qc (needed only when
            chunk qc becomes a key/value block: later than q/k)."""
            psv = psp.tile([P, 512], F32, tag="ps", name=f"psv{st4}")
            for dt in range(NDT):
                nc.tensor.matmul(psv, xpart(qc, dt)[:, bass.ts(st4, P)],
                                 wv_sb[:, dt],
                                 start=(dt == 0), stop=(dt == NDT - 1),
                                 skip_group_check=SKIP_A)
            psv4 = psv.rearrange("q (pp hh d) -> q pp hh d", pp=NPAIR, hh=2)
            nc.vector.tensor_copy(vaugt[qc][:, st4, :, 0:64],
                                  psv4[:, :, 0, :])
            nc.vector.tensor_copy(vaugt[qc][:, st4, :, 128:192],
                                  psv4[:, :, 1, :])

        def phase_b(qc, pp, eTp, qkp, pvp, scrp, fill_cb=None, need_cb=None,
                    head_cb=None):
            """attention for (chunk qc, pair pp): blocks j=0..4qc+3.

            With fill_cb, PV is software-pipelined one block behind QK and
            fill_cb(credit_ns) is invoked between QK(j) and PV(j-1) so the
            PE queue holds fill work exactly where it would otherwise stall
            waiting for exp(j-1) on ACT."""
            cw = slice(qc * 512, (qc + 1) * 512)
            pv0 = pvp.tile([65, 512], F32, tag="pv0", name="pv0")
            pv1 = pvp.tile([P, 512], F32, tag="pv1", name="pv1")
            nblk = 4 * qc + 4

            def emit_pv(j, c0, eTh):
                if "p" not in B_PARTS:
                    return
                vj = vaugt[j // 4][:, j % 4, pp]
                nc.tensor.matmul(pv0[:, c0:], vj[:, 0:65], eTh[0][:, c0:],
                                 start=(j == 0), stop=(j == nblk - 1),
                                 skip_group_check=True)
                nc.tensor.matmul(pv1[:, c0:], vj[:, 64:192],
                                 eTh[1][:, c0:],
                                 start=(j == 0), stop=(j == nblk - 1),
                                 skip_group_check=True)

            pend = None
            for j in range(nblk):
                c0 = max(0, (j - 4 * qc) * P)
                diag = j >= 4 * qc
                qk = qkp.tile([P, 2, 512], F32, tag="qk", name="qk")
                # two K=64 QK matmuls back-to-back on PE row-tiles
                # (0,0)/(64,0): they execute concurrently in 64x128 mode.
                mask_on = diag and "m" in B_PARTS and not MASK_DVE
                if "q" in B_PARTS:
                    for h in range(2):
                        hw = slice(64 * h, 64 * h + 64)
                        nc.tensor.matmul(
                            qk[:, h, c0:],
                            kTt[pp][j // 4][hw, (j % 4) * P:(j % 4 + 1) * P],
                            qTt[pp][qc][hw, c0:],
                            start=True, stop=not mask_on,
                            skip_group_check=True)
                if mask_on:
                    # causal mask add (-30000 where sk>sq): one K=128 matmul
                    # per head (two K=64 row-tiles would race on the bank).
                    for h in range(2):
                        nc.tensor.matmul(
                            qk[:, h, c0:c0 + P], id_sb, nm_sb,
                            start=False, stop=True,
                            skip_group_check=True)
                if need_cb is not None:
                    # force upcoming blocks' k/v projection units now: their
                    # matmuls ride behind QK(j) so the bias-add / vaug
                    # copies (DVE, possibly behind C2-copy backlog) are done
                    # before the consumer reads them.
                    if j + 3 < nblk:
                        need_cb("k", j + 3)
                    elif j + 2 < nblk:
                        need_cb("k", j + 2)
                    if j + 2 < nblk:
                        need_cb("v", j + 2)
                    elif j + 1 < nblk:
                        need_cb("v", j + 1)
                if EXP2:
                    # one ACT instruction covering BOTH heads: halves the
                    # 352-cycle per-instruction ACT overhead (ACT is the
                    # co-bottleneck engine at ~210us busy with 2 instrs/blk).
                    eT2 = eTp.tile([P, 2, 512], F16, tag="eT", name="eT2")
                    eTh = [eT2[:, 0], eT2[:, 1]]
                    if "e" in B_PARTS:
                        nc.scalar.activation(
                            eT2[:, :, c0:], qk[:, :, c0:],
                            mybir.ActivationFunctionType.Exp, scale=SCALE)
                    else:
                        nc.vector.memset(eT2.bitcast(F32), 1.0)
                else:
                    # per-head eT tiles: finer pool rotation (head h's buffer
                    # recycles as soon as ITS PV is done, not both heads')
                    eTh = [eTp.tile([P, 512], F16, tag="eT", name=f"eT{h}")
                           for h in range(2)]
                    if "e" in B_PARTS:
                        for h in range(2):
                            nc.scalar.activation(
                                eTh[h][:, c0:], qk[:, h, c0:],
                                mybir.ActivationFunctionType.Exp, scale=SCALE)
                    else:
                        for h in range(2):
                            nc.vector.memset(eTh[h].bitcast(F32), 1.0)
                if diag and "m" in B_PARTS and MASK_DVE:
                    # zero the causal triangle of eT post-exp on DVE
                    # (keeps the mask off the LDW-serialized PE).
                    for h in range(2):
                        nc.vector.tensor_mul(eTh[h][:, c0:c0 + P],
                                             eTh[h][:, c0:c0 + P], bm_sb)
                if fill_cb is None:
                    emit_pv(j, c0, eTh)
                else:
                    fill_cb((2 * (512 - c0) + 352) / 1.2
                            - FILL_FUDGE * 3 * (512 - c0) / 2.4)
                    if pend is not None:
                        emit_pv(*pend)
                    pend = (j, c0, eTh)
                    if j == 1 and head_cb is not None:
                        # previous stream's deferred C1 lands here: its
                        # DVE chain (recip etc.) completed while this
                        # stream's first blocks kept the PE busy, so the
                        # bc matmul no longer blocks the PE queue.
                        head_cb()
            if pend is not None:
                if fill_cb is not None:
                    fill_cb(C1_CREDIT)
                emit_pv(*pend)
            # 1/den straight out of PSUM on ACT (idle at stream end): two
            # Reciprocal activations replace the old DVE chain (row copies
            # -> approx recip -> fp32r rounding copy), shortening the
            # stream-end critical path into the bc matmul by ~1.5us.
            if ACT_RECIP:
                nc.scalar.activation(rct[pp][64:65, qc], pv0[64:65],
                                     mybir.ActivationFunctionType.Reciprocal)
                nc.scalar.activation(rct[pp][0:1, qc], pv1[0:1],
                                     mybir.ActivationFunctionType.Reciprocal)
            else:
                nc.scalar.activation(rct[pp][64:65, qc], pv0[64:65],
                                     mybir.ActivationFunctionType.Copy)
                nc.vector.tensor_copy(rct[pp][0:1, qc], pv1[0:1])
                rc = rct[pp]
                if APPROX_RECIP:
                    scr = scrp.tile([65, 512], F32, tag="scr", name="scr")
                    nc.vector.reciprocal_approx_fast(
                        scr, rc[:, qc].bitcast(F32))
                    # fp32r rounding copy on ACT: overlaps the outT copies
                    # queued on DVE, shortening the path into bc
                    nc.scalar.activation(rc[:, qc], scr,
                                         mybir.ActivationFunctionType.Copy)
                else:
                    nc.vector.reciprocal(rc[:, qc], rc[:, qc])
            nc.vector.tensor_copy(outT[0:64, pp, cw], pv0[0:64])
            nc.vector.tensor_copy(outT[64:128, pp, cw], pv1[64:128])

        def phase_c1(qc, pp, qkp, scrp):
            """normalize chunk qc of pair pp by 1/den, add bv.

            The reciprocal itself runs at the end of phase_b (ACT rct copy
            -> DVE recip) so only the bc broadcast + normalize remain here;
            this whole call is deferred into the next stream so the bc
            matmul never head-of-line blocks the PE on the recip chain."""
            cw = slice(qc * 512, (qc + 1) * 512)
            rc = rct[pp]
            bc = qkp.tile([P, 2, 512], F32, tag="qk", name="bc")[:, 0]
            nc.tensor.matmul(bc, sel_sb, rc[:, qc], start=True, stop=True)
            nc.vector.tensor_mul(outT[:, pp, cw], outT[:, pp, cw], bc)
            nc.vector.tensor_scalar_add(outT[:, pp, cw], outT[:, pp, cw],
                                        bv_sb[:, pp:pp + 1])

        def phase_c2_piece(st, outp, psp):
            """output projection for one 128-row seq-tile."""
            if True:
                sw = slice(st * P, (st + 1) * P)
                for dc in range(2):
                    ps = psp.tile([P, 512], F32, tag="ps", name="psC")
                    for p in range(NPAIR):
                        nc.tensor.matmul(ps, outT[:, p, sw],
                                         wout_sb[:, p, dc],
                                         start=(p == 0), stop=(p == NPAIR - 1),
                                         skip_group_check=SKIP_A)
                    ot = outp.tile([P, 512], F16, tag="ot")
                    nc.vector.tensor_copy(ot, ps)
                    nc.sync.dma_start(out[sw, dc * 512:(dc + 1) * 512], ot)

        # rc rows 1..63 are read (x0-weighted) by the bc matmul; memset once
        # so the PE never multiplies uninitialized SBUF (they stay ~1.0).
        # On gpsimd, NOT DVE: 4x2.2us at the head of the DVE queue would
        # delay the first qT bias-add (and thus the first QK) to ~9.5us.
        for pp in range(NPAIR):
            nc.gpsimd.memset(rct[pp].bitcast(F32), 1.0)

        def body_legacy(xtp, psp, eTp, qkp, pvp, outp, scrp):
            xts.clear()
            # head: pair 0's q/k + all v-tiles, so B(0,0) (and its exp on
            # ACT) starts after ~48 matmuls instead of ~96; the remaining
            # pairs' q/k slide into B(0)'s exp-latency gaps below.
            phase_a_piece(0, 0, xtp, psp)
            for st4 in range(4):
                phase_a_vpiece(0, st4, psp)
            for qc in range(NQC):
                # emission order = scheduler priority: after each attention
                # pair-stream, inline its normalize (C1), the next chunk's
                # projection piece, and the PREVIOUS chunk's out-projection
                # tile, so PE always has fill work during exp latency
                # (pool buffers recycle in emission order).
                for pp in range(NPAIR):
                    if "b" in PHASES:
                        phase_b(qc, pp, eTp, qkp, pvp, scrp)
                    if "c" in PHASES:
                        phase_c1(qc, pp, qkp, scrp)
                    if qc == 0 and pp + 1 < NPAIR:
                        phase_a_piece(0, pp + 1, xtp, psp)
                    if qc + 1 < NQC:
                        phase_a_piece(qc + 1, pp, xtp, psp)
                        phase_a_vpiece(qc + 1, pp, psp)
                    if "c" in PHASES and qc >= 1:
                        phase_c2_piece(4 * (qc - 1) + pp, outp, psp)
            if "c" in PHASES:
                for pp in range(NPAIR):
                    phase_c2_piece(12 + pp, outp, psp)

        def body_flat(xtp, psp, eTp, qkp, pvp, outp, scrp):
            """Block-granular fill scheduler.

            All non-attention PE work (projection units, out-proj halves)
            lives in a FIFO of ~1-2us units. Inside each attention stream,
            fill_cb drains units between QK(j) and PV(j-1) proportional to
            the stream's ACT-minus-PE deficit (debt accounting), so the PE
            never stalls on exp latency even in the ACT-bound late chunks.
            Units a stream depends on are force-emitted at its start (debt-
            clamped so forced prerequisites don't starve later fill)."""
            xts.clear()
            if N_WARM:
                # the first real matmul waits ~3.5us on the x/weight DMAs
                # (For_i barrier resets engines each iteration); junk
                # matmuls on a memset tile fill that PE idle and keep the
                # HAM clock at 8/8 across the iteration boundary.
                wt = scrp.tile([65, 512], F32, tag="scr", name="warm")
                nc.vector.memset(wt, 0.0)
                wps = qkp.tile([P, 2, 512], F32, tag="qk", name="warmps")
                for i in range(N_WARM):
                    nc.tensor.matmul(wps[:, 0], wt[0:64, 0:64].bitcast(F16),
                                     wt[0:64, 0:256].bitcast(F16),
                                     start=(i == 0), stop=(i == N_WARM - 1),
                                     skip_group_check=True)
            for qc in range(NQC):
                xload(qc, xtp)

            pending = {}
            fillq = deque()
            st8 = {"debt": 0.0, "reserve": False, "stream": 0}
            eligible_from = {}

            def make_qk_unit(which, qc, p):
                w_sb, b_sb, dst = ((wq_sb, bq_sb, qTt) if which == "q"
                                   else (wk_sb, bk_sb, kTt))

                def fn():
                    ps = psp.tile([P, 512], F32, tag="ps",
                                  name=f"ps{which}{p}")
                    for dt in range(NDT):
                        nc.tensor.matmul(ps, w_sb[:, dt, bass.ts(p, P)],
                                         xpart(qc, dt),
                                         start=(dt == 0),
                                         stop=(dt == NDT - 1),
                                         skip_group_check=SKIP_A)
                    nc.vector.tensor_scalar_add(dst[p][qc], ps,
                                                b_sb[:, p:p + 1])
                return fn

            def make_c2_unit(st, dc):
                def fn():
                    sw = slice(st * P, (st + 1) * P)
                    if st >= 4 * (NQC - 1) or st8.get("tailmode"):
                        # tail/reserved units run after attention is done:
                        # borrow the then-idle qkp pool — psp's two buffers
                        # are convoy-blocked behind pending evacuations.
                        ps = qkp.tile([P, 2, 512], F32, tag="qk",
                                      name="psC2")[:, 0]
                    else:
                        ps = psp.tile([P, 512], F32, tag="ps", name="psC")
                    for p in range(NPAIR):
                        nc.tensor.matmul(ps, outT[:, p, sw],
                                         wout_sb[:, p, dc],
                                         start=(p == 0),
                                         stop=(p == NPAIR - 1),
                                         skip_group_check=SKIP_A)
                    ot = outp.tile([P, 512], F16, tag="ot")
                    # PSUM->SBUF evacuation alternates ACT/DVE: on a single
                    # engine the copies convoy behind that queue's backlog
                    # (exp on ACT, stream-end copies on DVE) and the psp
                    # pool (bufs=2) then head-of-line blocks the PE.
                    if (st + dc) % 2 == 0:
                        nc.scalar.activation(ot, ps,
                                             mybir.ActivationFunctionType
                                             .Copy)
                    else:
                        nc.vector.tensor_copy(ot, ps)
                    nc.sync.dma_start(out[sw, dc * 512:(dc + 1) * 512], ot)
                return fn

            def add_unit(key, cost, fn, elig=0):
                pending[key] = (cost, fn)
                eligible_from[key] = elig
                fillq.append(key)

            for qc in range(NQC):
                for p in range(NPAIR):
                    add_unit(("q", qc, p), A_COST, make_qk_unit("q", qc, p))
                    add_unit(("k", qc, p), A_COST, make_qk_unit("k", qc, p))
                for st4 in range(4):
                    add_unit(("v", qc, st4), A_COST,
                             (lambda q, s: lambda: phase_a_vpiece(q, s, psp))
                             (qc, st4))

            def emit_unit(key):
                ent = pending.pop(key, None)
                if ent is None:
                    return 0.0
                if os.environ.get("KV2_DEBUG"):
                    print(f"emit {key} at stream {st8['stream']} "
                          f"debt {st8['debt']:.0f}")
                ent[1]()
                return ent[0]

            def navail():
                return sum(1 for k in fillq
                           if k in pending
                           and eligible_from[k] <= st8["stream"])

            def fill_cb(credit):
                st8["debt"] += credit
                while st8["debt"] > 0:
                    if st8["reserve"] and navail() <= RESERVE:
                        break
                    key = None
                    for k in fillq:
                        if k in pending and eligible_from[k] <= st8["stream"]:
                            key = k
                            break
                    if key is None:
                        break
                    fillq.remove(key)
                    st8["debt"] -= emit_unit(key)

            def force(keys):
                for key in keys:
                    st8["debt"] -= emit_unit(key)
                st8["debt"] = max(st8["debt"], -DEBT_CLAMP)

            pending_c1 = []

            def flush_c1():
                while pending_c1:
                    q0, p0 = pending_c1.pop(0)
                    if os.environ.get("KV2_DEBUG"):
                        print(f"c1 ({q0},{p0}) at stream {st8['stream']}")
                    phase_c1(q0, p0, qkp, scrp)
                    if p0 == NPAIR - 1:
                        # chunk q0's out-projection becomes fill, but only
                        # from stream (NQC-1, q0) on: released earlier it
                        # drains in regions that already have A-unit
                        # surplus, leaving the ACT-heavy last region dry.
                        for st in range(4 * q0, 4 * q0 + 4):
                            for dc in range(2):
                                add_unit(("c2", st, dc), C2_COST,
                                         make_c2_unit(st, dc),
                                         elig=(NQC - 1) * NPAIR + q0)

            for qc in range(NQC):
                for pp in range(NPAIR):
                    def need_cb(kind, j, _pp=pp):
                        if kind == "k":
                            force([("k", j // 4, _pp)])
                        else:
                            force([("v", j // 4, j % 4)])
                    force([("q", qc, pp), ("k", 0, pp),
                           ("v", 0, 0), ("v", 0, 1)])
                    st8["stream"] = qc * NPAIR + pp
                    # debt is per-stream pacing: carrying unmet credit from
                    # fill-dry regions would dump all gated units at once
                    st8["debt"] = min(st8["debt"], 0.0)
                    # during the LAST stream, hold back a few fill units so
                    # the PE has work while the final C1's DVE chain runs
                    st8["reserve"] = (qc, pp) == (NQC - 1, NPAIR - 1)
                    phase_b(qc, pp, eTp, qkp, pvp, scrp, fill_cb, need_cb,
                            head_cb=flush_c1)
                    pending_c1.append((qc, pp))
                    # prefetch the NEXT stream's q/k-chunk0 units so their
                    # bias-adds (DVE) finish before that stream's first QK
                    ns = qc * NPAIR + pp + 1
                    if ns < NQC * NPAIR:
                        force([("q", ns // NPAIR, ns % NPAIR),
                               ("k", 0, ns % NPAIR)])
            st8["reserve"] = False
            for key in list(fillq):  # reserved units bridge the C1 wait
                emit_unit(key)
            flush_c1()
            for key in list(fillq):
                emit_unit(key)

        body = body_flat if SCHED == "flat" else body_legacy

        with tc.tile_pool(name="xtp", bufs=4 if SCHED == "legacy" else 8) \
                as xtp, \
             tc.tile_pool(name="eTp", bufs=6) as eTp, \
             tc.tile_pool(name="scrp", bufs=2) as scrp, \
             tc.tile_pool(name="outp", bufs=3) as outp, \
             tc.tile_pool(name="psp", bufs=2, space="PSUM") as psp, \
             tc.tile_pool(name="qkp", bufs=2, space="PSUM") as qkp, \
             tc.tile_pool(name="pvp", bufs=1, space="PSUM") as pvp:
            if loop == "full":
                with tc.For_i(0, loop_n, 1):
                    body(xtp, psp, eTp, qkp, pvp, outp, scrp)
            else:
                body(xtp, psp, eTp, qkp, pvp, outp, scrp)
    nc.compile()
    _NC_CACHE[key] = nc
    return nc


def prep_core_inputs(x, W_qkv, b_qkv, W_out, core):
    b, g = core // 2, core % 2
    hs = slice(HEADS_LOC * g, HEADS_LOC * (g + 1))
    w3 = W_qkv.reshape(D_MODEL, 3, NHEAD, HEAD_DIM)
    b3 = b_qkv.reshape(3, NHEAD, HEAD_DIM)
    # bc-broadcast selector: den for (chunk, h0) sits at rc row 64,
    # (chunk, h1) at rc row 0; head h covers output partitions [64h, 64h+64).
    sel = np.zeros((65, P), np.float32)
    sel[64, :64] = 1.0
    sel[0, 64:] = 1.0
    tri = np.where(np.arange(P)[:, None] > np.arange(P)[None, :],
                   np.float16(-30000.0), np.float16(0.0))
    bm = np.where(np.arange(P)[:, None] > np.arange(P)[None, :],
                  np.float16(0.0), np.float16(1.0))
    return {
        "xT": np.ascontiguousarray(x[b].T).astype(np.float16),
        "wq": w3[:, 0, hs].reshape(D_MODEL, DL).astype(np.float16),
        "wk": w3[:, 1, hs].reshape(D_MODEL, DL).astype(np.float16),
        "wv": w3[:, 2, hs].reshape(D_MODEL, DL).astype(np.float16),
        "wout": np.ascontiguousarray(
            W_out.reshape(NHEAD, HEAD_DIM, D_MODEL)[hs].reshape(
                DL, D_MODEL)).astype(np.float16),
        "bq": np.ascontiguousarray(b3[0, hs].reshape(NPAIR, P).T,
                                   dtype=np.float32),
        "bk": np.ascontiguousarray(b3[1, hs].reshape(NPAIR, P).T,
                                   dtype=np.float32),
        "bv": np.ascontiguousarray(b3[2, hs].reshape(NPAIR, P).T,
                                   dtype=np.float32),
        "ident": np.eye(P, dtype=np.float16),
        "negmask": np.ascontiguousarray(tri, dtype=np.float16),
        "binmask": np.ascontiguousarray(bm, dtype=np.float16),
        "sel": sel,
        "onesv": np.ones((P, 4, NPAIR, 64), np.float16),
    }


def kernel(x, W_qkv, b_qkv, W_out, b_out):
    x = np.asarray(x, np.float32)
    W_qkv = np.asarray(W_qkv, np.float32)
    b_qkv = np.asarray(b_qkv, np.float32)
    W_out = np.asarray(W_out, np.float32)
    b_out = np.asarray(b_out, np.float32)
    nc = build_nc()
    in_maps = [prep_core_inputs(x, W_qkv, b_qkv, W_out, c)
               for c in range(N_CORES)]
    res = run_bass_kernel_spmd(nc, in_maps, core_ids=list(range(N_CORES)))
    out = np.empty((B, S, D_MODEL), np.float32)
    for b in range(B):
        out[b] = (res.results[2 * b]["out"].astype(np.float32)
                  + res.results[2 * b + 1]["out"].astype(np.float32) + b_out)
    return out



# revision 73
# speedup vs baseline: 1.0076x; 1.0076x over previous
"""Causal self-attention Trainium2 kernel (B=4, S=2048, D=1024, H=16, Dh=64).

Sharding: 8 cores = 4 batches x 2 head-groups (8 heads each). Each core
computes the qkv projection for its heads, causal attention, and a partial
output projection; the host sums the two partials per batch and adds b_out.

Design (all choices A/B-measured on hardware):
  - fp16 storage everywhere (x, W, qT/kT, v, eT, outT, DRAM out); PSUM
    stays fp32. fp16 streams ~2-5x faster per matmul than fp32r/bf16 on
    this toolchain and has no N>=256 restriction, so diagonal blocks use
    exact widths (512/384/256/128).
  - weights loaded once, SBUF-resident; one-time loads ride the gpsimd
    SWDGE queue so x loads on the sync queue are never blocked.
  - fused pipeline, emission order = scheduler priority: after each
    attention pair-stream emit its softmax-normalize (C1), the next
    chunk's projection piece, and the PREVIOUS chunk's out-projection
    tile, so the PE always has fill work during exp latency. At the
    chunk head, pair 0's q/k plus all v-tiles go first so exp starts
    early; remaining pairs' q/k slide into the exp gaps.
  - QK per head = K=64 matmul into its own PSUM bank (auto row-tiles
    (0,0)/(64,0); same-bank row-tile pairs RACE - never do that).
  - exp as two per-head ACT instructions into per-head eT tiles (finer
    chain granularity lets PV(h0) start under exp(h1)).
  - causal mask = post-exp DVE multiply by a 0/1 triangle (keeps the
    mask off the LDW-serialized PE).
  - softmax denominators via the ones-column-in-v trick: PV emits den at
    a known partition; 1/den via reciprocal_approx_fast, broadcast to
    128 partitions by a tiny selector matmul, applied with the bv bias
    fold (attn sums to 1, so out = pv/den + bv needs no v pre-bias).
"""
import os
import numpy as np
from collections import deque
from contextlib import ExitStack

GPSIMD_DMA = os.environ.get("KV2_GPSIMD_DMA", "1") == "1"
EXP2 = os.environ.get("KV2_EXP2", "1") == "1"
SCHED = os.environ.get("KV2_SCHED", "flat")  # flat | legacy
FILL_FUDGE = float(os.environ.get("KV2_FUDGE", "1.15"))
C1_CREDIT = float(os.environ.get("KV2_C1_CREDIT", "1500"))
DEBT_CLAMP = float(os.environ.get("KV2_CLAMP", "2000"))
RESERVE = int(os.environ.get("KV2_RESERVE", "3"))
A_COST = FILL_FUDGE * 8 * 512 / 2.4      # one q/k/v projection unit
C2_COST = FILL_FUDGE * 4 * 512 / 2.4     # one out-proj half-piece
EXP_SPAN = os.environ.get("KV2_EXP_SPAN", "0") == "1"
PHASES = os.environ.get("KV2_PHASES", "abc")
B_PARTS = os.environ.get("KV2_B_PARTS", "qmep")  # q=QK m=mask e=exp p=PV
SKIP_A = os.environ.get("KV2_SKIP_A", "0") == "1"
# "direct" (skip the rounding copy) passes CoreSim but the neuronxcc BIR
# verifier rejects a non-rounded fp32r matmult operand — keep the copy.
_AR = os.environ.get("KV2_APPROX_RECIP", "1")
APPROX_RECIP = _AR if _AR == "direct" else _AR == "1"
# ACT Reciprocal is hard-blocked by bass (accuracy); keep the DVE recip.
ACT_RECIP = os.environ.get("KV2_ACT_RECIP", "0") == "1"
N_WARM = int(os.environ.get("KV2_WARM", "8"))
HEAD_KICK = float(os.environ.get("KV2_HEAD_KICK", "0"))
TAILQKP = os.environ.get("KV2_TAILQKP", "0") == "1"
MASK_DVE = os.environ.get("KV2_MASK_DVE", "1") == "1"

import concourse.bass as bass
import concourse.tile as tile
from concourse import bacc, mybir
from concourse.bass_utils import run_bass_kernel_spmd

P = 128
D_MODEL = 1024
NHEAD = 16
HEAD_DIM = 64
B = 4
S = 2048
N_CORES = 8
HEADS_LOC = 8           # heads per core
NPAIR = HEADS_LOC // 2  # head pairs per core
DL = HEADS_LOC * HEAD_DIM  # local dims = 512
NDT = D_MODEL // P      # 8 d-tiles
NQC = S // 512          # sq chunks of 512
NST = S // P            # s tiles of 128
VSLOT = 192             # per-pair v slot: v_h0(64) | ones(64) | v_h1(64)
F32 = mybir.dt.float32
F32R = mybir.dt.float32r
F16 = mybir.dt.float16
SCALE = HEAD_DIM ** -0.5

_NC_CACHE = {}


def build_nc(loop=None, loop_n=17):
    key = (loop, loop_n if loop else 0)
    if key in _NC_CACHE:
        return _NC_CACHE[key]
    nc = bacc.Bacc("TRN2", target_bir_lowering=False, debug=False,
                   num_devices=N_CORES)
    xT = nc.dram_tensor("xT", [D_MODEL, S], F16, kind="ExternalInput").ap()
    wq = nc.dram_tensor("wq", [D_MODEL, DL], F16, kind="ExternalInput").ap()
    wk = nc.dram_tensor("wk", [D_MODEL, DL], F16, kind="ExternalInput").ap()
    wv = nc.dram_tensor("wv", [D_MODEL, DL], F16, kind="ExternalInput").ap()
    wout = nc.dram_tensor("wout", [DL, D_MODEL], F16, kind="ExternalInput").ap()
    bq = nc.dram_tensor("bq", [P, NPAIR], F32, kind="ExternalInput").ap()
    bk = nc.dram_tensor("bk", [P, NPAIR], F32, kind="ExternalInput").ap()
    bv = nc.dram_tensor("bv", [P, NPAIR], F32, kind="ExternalInput").ap()
    ident = nc.dram_tensor("ident", [P, P], F16, kind="ExternalInput").ap()
    negmask = nc.dram_tensor("negmask", [P, P], F16, kind="ExternalInput").ap()
    binmask = nc.dram_tensor("binmask", [P, P], F16, kind="ExternalInput").ap()
    sel = nc.dram_tensor("sel", [65, P], F32R, kind="ExternalInput").ap()
    onesv = nc.dram_tensor("onesv", [P, 4, NPAIR, 64], F16,
                           kind="ExternalInput").ap()
    out = nc.dram_tensor("out", [S, D_MODEL], F16, kind="ExternalOutput").ap()

    wqr = wq.rearrange("(a p) n -> p a n", p=P)
    wkr = wk.rearrange("(a p) n -> p a n", p=P)
    wvr = wv.rearrange("(a p) n -> p a n", p=P)
    woutr = wout.rearrange("(a p) (b n) -> p a b n", p=P, n=512)
    xTr = xT.rearrange("(a p) s -> p a s", p=P)

    with tile.TileContext(nc) as tc, ExitStack() as ctx, \
         nc.allow_low_precision(reason="fp16 storage is within tolerance"):
        # ---- persistent SBUF ----
        persist = ctx.enter_context(tc.tile_pool(name="persist", bufs=1))
        qTt = [[persist.tile([P, 512], F16, tag=f"qT{p}_{qc}",
                             name=f"qT{p}_{qc}")
                for qc in range(NQC)] for p in range(NPAIR)]
        kTt = [[persist.tile([P, 512], F16, tag=f"kT{p}_{qc}",
                             name=f"kT{p}_{qc}")
                for qc in range(NQC)] for p in range(NPAIR)]
        vaugt = [persist.tile([P, 4, NPAIR, VSLOT], F16, tag=f"v{qc}",
                              name=f"vaug{qc}")
                 for qc in range(NQC)]
        outT = persist.tile([P, NPAIR, S], F16, tag="outT", name="outT")
        rct = [persist.tile([65, NQC, 512], F32R, tag=f"rc{pp}",
                            name=f"rc{pp}") for pp in range(NPAIR)]

        consts = ctx.enter_context(tc.tile_pool(name="consts", bufs=1))
        wq_sb = consts.tile([P, NDT, DL], F16, tag="wq")
        wk_sb = consts.tile([P, NDT, DL], F16, tag="wk")
        wv_sb = consts.tile([P, NDT, DL], F16, tag="wv")
        wout_sb = consts.tile([P, NPAIR, 2, 512], F16, tag="wout")
        bq_sb = consts.tile([P, NPAIR], F32, tag="bq")
        bk_sb = consts.tile([P, NPAIR], F32, tag="bk")
        bv_sb = consts.tile([P, NPAIR], F32, tag="bv")
        id_sb = consts.tile([P, P], F16, tag="ident")
        nm_sb = consts.tile([P, P], F16, tag="negmask")
        bm_sb = consts.tile([P, P], F16, tag="binmask")
        sel_sb = consts.tile([65, P], F32R, tag="sel")
        # one-time loads on the gpsimd SWDGE queue, in PARALLEL with the x
        # chunk loads on the sync queue. Order = first-use order: pair-0
        # slices of wq/wk lead (first projection matmul ~1.6us in), then
        # wv + chunk-0 ones (PV(0) needs them ~7us in), then the rest.
        dmae = nc.gpsimd if GPSIMD_DMA else nc.sync
        # pair-0 wq/bq on the sync queue (starts fastest, ahead of the x
        # loads emitted by the body) so the first matmul isn't gated on
        # the gpsimd queue's startup latency.
        nc.sync.dma_start(wq_sb[:, :, 0:P], wqr[:, :, 0:P])
        nc.sync.dma_start(bq_sb, bq)
        dmae.dma_start(wk_sb[:, :, 0:P], wkr[:, :, 0:P])
        dmae.dma_start(bk_sb, bk)
        dmae.dma_start(wv_sb, wvr)
        dmae.dma_start(vaugt[0][:, :, :, 64:128], onesv)
        dmae.dma_start(bv_sb, bv)
        dmae.dma_start(wq_sb[:, :, P:], wqr[:, :, P:])
        dmae.dma_start(wk_sb[:, :, P:], wkr[:, :, P:])
        for qc in range(1, NQC):
            dmae.dma_start(vaugt[qc][:, :, :, 64:128], onesv)
        dmae.dma_start(id_sb, ident)
        dmae.dma_start(nm_sb, negmask)
        dmae.dma_start(bm_sb, binmask)
        dmae.dma_start(sel_sb, sel)
        dmae.dma_start(wout_sb, woutr)

        xts = {}

        def xload(qc, xtp):
            """x chunk qc as TWO tiles (dt 0..3 / 4..7) so consumers of the
            first half don't wait on the whole chunk's DMA. Chunk 0 rides
            the scalar/vector queues: in PARALLEL with wq on sync, so the
            first projection matmul starts ~2us earlier (the fixed
            DMA-to-semaphore latency makes serial queue order costly)."""
            cw = slice(qc * 512, (qc + 1) * 512)
            xa = xtp.tile([P, 4, 512], F16, tag="xt", name=f"x{qc}a")
            xb = xtp.tile([P, 4, 512], F16, tag="xt", name=f"x{qc}b")
            ea = nc.scalar if qc == 0 else nc.sync
            ea.dma_start(xa, xTr[:, 0:4, cw])
            ea.dma_start(xb, xTr[:, 4:8, cw])
            xts[qc] = (xa, xb)

        def xpart(qc, dt):
            xa, xb = xts[qc]
            return xa[:, dt] if dt < 4 else xb[:, dt - 4]

        def phase_a_piece(qc, p, xtp, psp):
            """1/4 of chunk qc's projection: qT/kT for pair p, v for st4=p."""
            if qc not in xts:
                xload(qc, xtp)
            psq = psp.tile([P, 512], F32, tag="ps", name=f"psq{p}")
            for dt in range(NDT):
                nc.tensor.matmul(psq, wq_sb[:, dt, bass.ts(p, P)],
                                 xpart(qc, dt),
                                 start=(dt == 0), stop=(dt == NDT - 1),
                                 skip_group_check=SKIP_A)
            nc.vector.tensor_scalar_add(qTt[p][qc], psq, bq_sb[:, p:p + 1])
            psk = psp.tile([P, 512], F32, tag="ps", name=f"psk{p}")
            for dt in range(NDT):
                nc.tensor.matmul(psk, wk_sb[:, dt, bass.ts(p, P)],
                                 xpart(qc, dt),
                                 start=(dt == 0), stop=(dt == NDT - 1),
                                 skip_group_check=SKIP_A)
            nc.vector.tensor_scalar_add(kTt[p][qc], psk, bk_sb[:, p:p + 1])

        def phase_a_vpiece(qc, st4, psp):
            """v-projection for seq-tile st4 of chunk qc (needed only when
            chunk qc becomes a key/value block: later than q/k)."""
            psv = psp.tile([P, 512], F32, tag="ps", name=f"psv{st4}")
            for dt in range(NDT):
                nc.tensor.matmul(psv, xpart(qc, dt)[:, bass.ts(st4, P)],
                                 wv_sb[:, dt],
                                 start=(dt == 0), stop=(dt == NDT - 1),
                                 skip_group_check=SKIP_A)
            psv4 = psv.rearrange("q (pp hh d) -> q pp hh d", pp=NPAIR, hh=2)
            nc.vector.tensor_copy(vaugt[qc][:, st4, :, 0:64],
                                  psv4[:, :, 0, :])
            nc.vector.tensor_copy(vaugt[qc][:, st4, :, 128:192],
                                  psv4[:, :, 1, :])

        def phase_b(qc, pp, eTp, qkp, pvp, scrp, fill_cb=None, need_cb=None,
                    head_cb=None):
            """attention for (chunk qc, pair pp): blocks j=0..4qc+3.

            With fill_cb, PV is software-pipelined one block behind QK and
            fill_cb(credit_ns) is invoked between QK(j) and PV(j-1) so the
            PE queue holds fill work exactly where it would otherwise stall
            waiting for exp(j-1) on ACT."""
            cw = slice(qc * 512, (qc + 1) * 512)
            pv0 = pvp.tile([65, 512], F32, tag="pv0", name="pv0")
            pv1 = pvp.tile([P, 512], F32, tag="pv1", name="pv1")
            nblk = 4 * qc + 4

            def emit_pv(j, c0, eTh):
                if "p" not in B_PARTS:
                    return
                vj = vaugt[j // 4][:, j % 4, pp]
                nc.tensor.matmul(pv0[:, c0:], vj[:, 0:65], eTh[0][:, c0:],
                                 start=(j == 0), stop=(j == nblk - 1),
                                 skip_group_check=True)
                nc.tensor.matmul(pv1[:, c0:], vj[:, 64:192],
                                 eTh[1][:, c0:],
                                 start=(j == 0), stop=(j == nblk - 1),
                                 skip_group_check=True)

            pend = None
            for j in range(nblk):
                c0 = max(0, (j - 4 * qc) * P)
                diag = j >= 4 * qc
                qk = qkp.tile([P, 2, 512], F32, tag="qk", name="qk")
                # two K=64 QK matmuls back-to-back on PE row-tiles
                # (0,0)/(64,0): they execute concurrently in 64x128 mode.
                mask_on = diag and "m" in B_PARTS and not MASK_DVE
                if "q" in B_PARTS:
                    for h in range(2):
                        hw = slice(64 * h, 64 * h + 64)
                        nc.tensor.matmul(
                            qk[:, h, c0:],
                            kTt[pp][j // 4][hw, (j % 4) * P:(j % 4 + 1) * P],
                            qTt[pp][qc][hw, c0:],
                            start=True, stop=not mask_on,
                            skip_group_check=True)
                if mask_on:
                    # causal mask add (-30000 where sk>sq): one K=128 matmul
                    # per head (two K=64 row-tiles would race on the bank).
                    for h in range(2):
                        nc.tensor.matmul(
                            qk[:, h, c0:c0 + P], id_sb, nm_sb,
                            start=False, stop=True,
                            skip_group_check=True)
                if need_cb is not None:
                    # force upcoming blocks' k/v projection units now: their
                    # matmuls ride behind QK(j) so the bias-add / vaug
                    # copies (DVE, possibly behind C2-copy backlog) are done
                    # before the consumer reads them.
                    if j + 3 < nblk:
                        need_cb("k", j + 3)
                    elif j + 2 < nblk:
                        need_cb("k", j + 2)
                    if j + 2 < nblk:
                        need_cb("v", j + 2)
                    elif j + 1 < nblk:
                        need_cb("v", j + 1)
                if EXP2:
                    # one ACT instruction covering BOTH heads: halves the
                    # 352-cycle per-instruction ACT overhead (ACT is the
                    # co-bottleneck engine at ~210us busy with 2 instrs/blk).
                    eT2 = eTp.tile([P, 2, 512], F16, tag="eT", name="eT2")
                    eTh = [eT2[:, 0], eT2[:, 1]]
                    if "e" in B_PARTS:
                        nc.scalar.activation(
                            eT2[:, :, c0:], qk[:, :, c0:],
                            mybir.ActivationFunctionType.Exp, scale=SCALE)
                    else:
                        nc.vector.memset(eT2.bitcast(F32), 1.0)
                else:
                    # per-head eT tiles: finer pool rotation (head h's buffer
                    # recycles as soon as ITS PV is done, not both heads')
                    eTh = [eTp.tile([P, 512], F16, tag="eT", name=f"eT{h}")
                           for h in range(2)]
                    if "e" in B_PARTS:
                        for h in range(2):
                            nc.scalar.activation(
                                eTh[h][:, c0:], qk[:, h, c0:],
                                mybir.ActivationFunctionType.Exp, scale=SCALE)
                    else:
                        for h in range(2):
                            nc.vector.memset(eTh[h].bitcast(F32), 1.0)
                if diag and "m" in B_PARTS and MASK_DVE:
                    # zero the causal triangle of eT post-exp on DVE
                    # (keeps the mask off the LDW-serialized PE).
                    for h in range(2):
                        nc.vector.tensor_mul(eTh[h][:, c0:c0 + P],
                                             eTh[h][:, c0:c0 + P], bm_sb)
                if fill_cb is None:
                    emit_pv(j, c0, eTh)
                else:
                    fill_cb((2 * (512 - c0) + 352) / 1.2
                            - FILL_FUDGE * 3 * (512 - c0) / 2.4)
                    if pend is not None:
                        emit_pv(*pend)
                    pend = (j, c0, eTh)
                    if j == 1 and head_cb is not None:
                        # previous stream's deferred C1 lands here: its
                        # DVE chain (recip etc.) completed while this
                        # stream's first blocks kept the PE busy, so the
                        # bc matmul no longer blocks the PE queue.
                        head_cb()
            if pend is not None:
                if fill_cb is not None:
                    fill_cb(C1_CREDIT)
                emit_pv(*pend)
            # 1/den straight out of PSUM on ACT (idle at stream end): two
            # Reciprocal activations replace the old DVE chain (row copies
            # -> approx recip -> fp32r rounding copy), shortening the
            # stream-end critical path into the bc matmul by ~1.5us.
            if ACT_RECIP:
                nc.scalar.activation(rct[pp][64:65, qc], pv0[64:65],
                                     mybir.ActivationFunctionType.Reciprocal)
                nc.scalar.activation(rct[pp][0:1, qc], pv1[0:1],
                                     mybir.ActivationFunctionType.Reciprocal)
            else:
                nc.scalar.activation(rct[pp][64:65, qc], pv0[64:65],
                                     mybir.ActivationFunctionType.Copy)
                nc.vector.tensor_copy(rct[pp][0:1, qc], pv1[0:1])
                rc = rct[pp]
                if APPROX_RECIP:
                    scr = scrp.tile([65, 512], F32, tag="scr", name="scr")
                    nc.vector.reciprocal_approx_fast(
                        scr, rc[:, qc].bitcast(F32))
                    # fp32r rounding copy on ACT: overlaps the outT copies
                    # queued on DVE, shortening the path into bc
                    nc.scalar.activation(rc[:, qc], scr,
                                         mybir.ActivationFunctionType.Copy)
                else:
                    nc.vector.reciprocal(rc[:, qc], rc[:, qc])
            nc.vector.tensor_copy(outT[0:64, pp, cw], pv0[0:64])
            nc.vector.tensor_copy(outT[64:128, pp, cw], pv1[64:128])

        def phase_c1(qc, pp, qkp, scrp):
            """normalize chunk qc of pair pp by 1/den, add bv.

            The reciprocal itself runs at the end of phase_b (ACT rct copy
            -> DVE recip) so only the bc broadcast + normalize remain here;
            this whole call is deferred into the next stream so the bc
            matmul never head-of-line blocks the PE on the recip chain."""
            cw = slice(qc * 512, (qc + 1) * 512)
            rc = rct[pp]
            bc = qkp.tile([P, 2, 512], F32, tag="qk", name="bc")[:, 0]
            nc.tensor.matmul(bc, sel_sb, rc[:, qc], start=True, stop=True)
            nc.vector.tensor_mul(outT[:, pp, cw], outT[:, pp, cw], bc)
            nc.vector.tensor_scalar_add(outT[:, pp, cw], outT[:, pp, cw],
                                        bv_sb[:, pp:pp + 1])

        def phase_c2_piece(st, outp, psp):
            """output projection for one 128-row seq-tile."""
            if True:
                sw = slice(st * P, (st + 1) * P)
                for dc in range(2):
                    ps = psp.tile([P, 512], F32, tag="ps", name="psC")
                    for p in range(NPAIR):
                        nc.tensor.matmul(ps, outT[:, p, sw],
                                         wout_sb[:, p, dc],
                                         start=(p == 0), stop=(p == NPAIR - 1),
                                         skip_group_check=SKIP_A)
                    ot = outp.tile([P, 512], F16, tag="ot")
                    nc.vector.tensor_copy(ot, ps)
                    nc.sync.dma_start(out[sw, dc * 512:(dc + 1) * 512], ot)

        # rc rows 1..63 are read (x0-weighted) by the bc matmul; memset once
        # so the PE never multiplies uninitialized SBUF (they stay ~1.0).
        # On gpsimd, NOT DVE: 4x2.2us at the head of the DVE queue would
        # delay the first qT bias-add (and thus the first QK) to ~9.5us.
        for pp in range(NPAIR):
            nc.gpsimd.memset(rct[pp].bitcast(F32), 1.0)

        def body_legacy(xtp, psp, eTp, qkp, pvp, outp, scrp):
            xts.clear()
            # head: pair 0's q/k + all v-tiles, so B(0,0) (and its exp on
            # ACT) starts after ~48 matmuls instead of ~96; the remaining
            # pairs' q/k slide into B(0)'s exp-latency gaps below.
            phase_a_piece(0, 0, xtp, psp)
            for st4 in range(4):
                phase_a_vpiece(0, st4, psp)
            for qc in range(NQC):
                # emission order = scheduler priority: after each attention
                # pair-stream, inline its normalize (C1), the next chunk's
                # projection piece, and the PREVIOUS chunk's out-projection
                # tile, so PE always has fill work during exp latency
                # (pool buffers recycle in emission order).
                for pp in range(NPAIR):
                    if "b" in PHASES:
                        phase_b(qc, pp, eTp, qkp, pvp, scrp)
                    if "c" in PHASES:
                        phase_c1(qc, pp, qkp, scrp)
                    if qc == 0 and pp + 1 < NPAIR:
                        phase_a_piece(0, pp + 1, xtp, psp)
                    if qc + 1 < NQC:
                        phase_a_piece(qc + 1, pp, xtp, psp)
                        phase_a_vpiece(qc + 1, pp, psp)
                    if "c" in PHASES and qc >= 1:
                        phase_c2_piece(4 * (qc - 1) + pp, outp, psp)
            if "c" in PHASES:
                for pp in range(NPAIR):
                    phase_c2_piece(12 + pp, outp, psp)

        def body_flat(xtp, psp, eTp, qkp, pvp, outp, scrp):
            """Block-granular fill scheduler.

            All non-attention PE work (projection units, out-proj halves)
            lives in a FIFO of ~1-2us units. Inside each attention stream,
            fill_cb drains units between QK(j) and PV(j-1) proportional to
            the stream's ACT-minus-PE deficit (debt accounting), so the PE
            never stalls on exp latency even in the ACT-bound late chunks.
            Units a stream depends on are force-emitted at its start (debt-
            clamped so forced prerequisites don't starve later fill)."""
            xts.clear()
            if N_WARM:
                # the first real matmul waits ~3.5us on the x/weight DMAs
                # (For_i barrier resets engines each iteration); junk
                # matmuls on a memset tile fill that PE idle and keep the
                # HAM clock at 8/8 across the iteration boundary.
                wt = scrp.tile([65, 512], F32, tag="scr", name="warm")
                nc.vector.memset(wt, 0.0)
                wps = qkp.tile([P, 2, 512], F32, tag="qk", name="warmps")
                for i in range(N_WARM):
                    nc.tensor.matmul(wps[:, 0], wt[0:64, 0:64].bitcast(F16),
                                     wt[0:64, 0:256].bitcast(F16),
                                     start=(i == 0), stop=(i == N_WARM - 1),
                                     skip_group_check=True)
            for qc in range(NQC):
                xload(qc, xtp)

            pending = {}
            fillq = deque()
            st8 = {"debt": 0.0, "reserve": False, "stream": 0}
            eligible_from = {}

            def make_qk_unit(which, qc, p):
                w_sb, b_sb, dst = ((wq_sb, bq_sb, qTt) if which == "q"
                                   else (wk_sb, bk_sb, kTt))

                def fn():
                    ps = psp.tile([P, 512], F32, tag="ps",
                                  name=f"ps{which}{p}")
                    for dt in range(NDT):
                        nc.tensor.matmul(ps, w_sb[:, dt, bass.ts(p, P)],
                                         xpart(qc, dt),
                                         start=(dt == 0),
                                         stop=(dt == NDT - 1),
                                         skip_group_check=SKIP_A)
                    nc.vector.tensor_scalar_add(dst[p][qc], ps,
                                                b_sb[:, p:p + 1])
                return fn

            def make_c2_unit(st, dc):
                def fn():
                    sw = slice(st * P, (st + 1) * P)
                    if st >= 4 * (NQC - 1) or st8.get("tailmode"):
                        # tail/reserved units run after attention is done:
                        # borrow the then-idle qkp pool — psp's two buffers
                        # are convoy-blocked behind pending evacuations.
                        ps = qkp.tile([P, 2, 512], F32, tag="qk",
                                      name="psC2")[:, 0]
                    else:
                        ps = psp.tile([P, 512], F32, tag="ps", name="psC")
                    for p in range(NPAIR):
                        nc.tensor.matmul(ps, outT[:, p, sw],
                                         wout_sb[:, p, dc],
                                         start=(p == 0),
                                         stop=(p == NPAIR - 1),
                                         skip_group_check=SKIP_A)
                    ot = outp.tile([P, 512], F16, tag="ot")
                    # PSUM->SBUF evacuation alternates ACT/DVE: on a single
                    # engine the copies convoy behind that queue's backlog
                    # (exp on ACT, stream-end copies on DVE) and the psp
                    # pool (bufs=2) then head-of-line blocks the PE.
                    if (st + dc) % 2 == 0:
                        nc.scalar.activation(ot, ps,
                                             mybir.ActivationFunctionType
                                             .Copy)
                    else:
                        nc.vector.tensor_copy(ot, ps)
                    nc.sync.dma_start(out[sw, dc * 512:(dc + 1) * 512], ot)
                return fn

            def add_unit(key, cost, fn, elig=0):
                pending[key] = (cost, fn)
                eligible_from[key] = elig
                fillq.append(key)

            for qc in range(NQC):
                for p in range(NPAIR):
                    add_unit(("q", qc, p), A_COST, make_qk_unit("q", qc, p))
                    add_unit(("k", qc, p), A_COST, make_qk_unit("k", qc, p))
                for st4 in range(4):
                    add_unit(("v", qc, st4), A_COST,
                             (lambda q, s: lambda: phase_a_vpiece(q, s, psp))
                             (qc, st4))

            def emit_unit(key):
                ent = pending.pop(key, None)
                if ent is None:
                    return 0.0
                if os.environ.get("KV2_DEBUG"):
                    print(f"emit {key} at stream {st8['stream']} "
                          f"debt {st8['debt']:.0f}")
                ent[1]()
                return ent[0]

            def navail():
                return sum(1 for k in fillq
                           if k in pending
                           and eligible_from[k] <= st8["stream"])

            def fill_cb(credit):
                st8["debt"] += credit
                while st8["debt"] > 0:
                    if st8["reserve"] and navail() <= RESERVE:
                        break
                    key = None
                    for k in fillq:
                        if k in pending and eligible_from[k] <= st8["stream"]:
                            key = k
                            break
                    if key is None:
                        break
                    fillq.remove(key)
                    st8["debt"] -= emit_unit(key)

            def force(keys):
                for key in keys:
                    st8["debt"] -= emit_unit(key)
                st8["debt"] = max(st8["debt"], -DEBT_CLAMP)

            pending_c1 = []

            def flush_c1():
                while pending_c1:
                    q0, p0 = pending_c1.pop(0)
                    if os.environ.get("KV2_DEBUG"):
                        print(f"c1 ({q0},{p0}) at stream {st8['stream']}")
                    phase_c1(q0, p0, qkp, scrp)
                    if p0 == NPAIR - 1:
                        # chunk q0's out-projection becomes fill, but only
                        # from stream (NQC-1, q0) on: released earlier it
                        # drains in regions that already have A-unit
                        # surplus, leaving the ACT-heavy last region dry.
                        for st in range(4 * q0, 4 * q0 + 4):
                            for dc in range(2):
                                add_unit(("c2", st, dc), C2_COST,
                                         make_c2_unit(st, dc),
                                         elig=(NQC - 1) * NPAIR + q0)

            for qc in range(NQC):
                for pp in range(NPAIR):
                    def need_cb(kind, j, _pp=pp):
                        if kind == "k":
                            force([("k", j // 4, _pp)])
                        else:
                            force([("v", j // 4, j % 4)])
                    force([("q", qc, pp), ("k", 0, pp),
                           ("v", 0, 0), ("v", 0, 1)])
                    st8["stream"] = qc * NPAIR + pp
                    # per-stream pacing reset, with a positive kick: one
                    # fill unit lands between QK(0/1) and PV(0), covering
                    # exp(0)'s latency at the stream head (was a 0.8us PE
                    # hole); carrying a big positive debt across streams
                    # would instead dump all gated units at once.
                    st8["debt"] = min(st8["debt"], HEAD_KICK)
                    # during the LAST stream, hold back a few fill units so
                    # the PE has work while the final C1's DVE chain runs
                    st8["reserve"] = (qc, pp) == (NQC - 1, NPAIR - 1)
                    phase_b(qc, pp, eTp, qkp, pvp, scrp, fill_cb, need_cb,
                            head_cb=flush_c1)
                    pending_c1.append((qc, pp))
                    # prefetch the NEXT stream's q/k-chunk0 units so their
                    # bias-adds (DVE) finish before that stream's first QK
                    ns = qc * NPAIR + pp + 1
                    if ns < NQC * NPAIR:
                        force([("q", ns // NPAIR, ns % NPAIR),
                               ("k", 0, ns % NPAIR)])
            st8["reserve"] = False
            st8["tailmode"] = TAILQKP  # reserved units borrow qkp PSUM
            for key in list(fillq):  # reserved units bridge the C1 wait
                emit_unit(key)
            flush_c1()
            for key in list(fillq):
                emit_unit(key)

        body = body_flat if SCHED == "flat" else body_legacy

        with tc.tile_pool(name="xtp", bufs=4 if SCHED == "legacy" else 8) \
                as xtp, \
             tc.tile_pool(name="eTp", bufs=6) as eTp, \
             tc.tile_pool(name="scrp", bufs=2) as scrp, \
             tc.tile_pool(name="outp", bufs=3) as outp, \
             tc.tile_pool(name="psp", bufs=2, space="PSUM") as psp, \
             tc.tile_pool(name="qkp", bufs=2, space="PSUM") as qkp, \
             tc.tile_pool(name="pvp", bufs=1, space="PSUM") as pvp:
            if loop == "full":
                with tc.For_i(0, loop_n, 1):
                    body(xtp, psp, eTp, qkp, pvp, outp, scrp)
            else:
                body(xtp, psp, eTp, qkp, pvp, outp, scrp)
    nc.compile()
    _NC_CACHE[key] = nc
    return nc


def prep_core_inputs(x, W_qkv, b_qkv, W_out, core):
    b, g = core // 2, core % 2
    hs = slice(HEADS_LOC * g, HEADS_LOC * (g + 1))
    w3 = W_qkv.reshape(D_MODEL, 3, NHEAD, HEAD_DIM)
    b3 = b_qkv.reshape(3, NHEAD, HEAD_DIM)
    # bc-broadcast selector: den for (chunk, h0) sits at rc row 64,
    # (chunk, h1) at rc row 0; head h covers output partitions [64h, 64h+64).
    sel = np.zeros((65, P), np.float32)
    sel[64, :64] = 1.0
    sel[0, 64:] = 1.0
    tri = np.where(np.arange(P)[:, None] > np.arange(P)[None, :],
                   np.float16(-30000.0), np.float16(0.0))
    bm = np.where(np.arange(P)[:, None] > np.arange(P)[None, :],
                  np.float16(0.0), np.float16(1.0))
    return {
        "xT": np.ascontiguousarray(x[b].T).astype(np.float16),
        "wq": w3[:, 0, hs].reshape(D_MODEL, DL).astype(np.float16),
        "wk": w3[:, 1, hs].reshape(D_MODEL, DL).astype(np.float16),
        "wv": w3[:, 2, hs].reshape(D_MODEL, DL).astype(np.float16),
        "wout": np.ascontiguousarray(
            W_out.reshape(NHEAD, HEAD_DIM, D_MODEL)[hs].reshape(
                DL, D_MODEL)).astype(np.float16),
        "bq": np.ascontiguousarray(b3[0, hs].reshape(NPAIR, P).T,
                                   dtype=np.float32),
        "bk": np.ascontiguousarray(b3[1, hs].reshape(NPAIR, P).T,
                                   dtype=np.float32),
        "bv": np.ascontiguousarray(b3[2, hs].reshape(NPAIR, P).T,
                                   dtype=np.float32),
        "ident": np.eye(P, dtype=np.float16),
        "negmask": np.ascontiguousarray(tri, dtype=np.float16),
        "binmask": np.ascontiguousarray(bm, dtype=np.float16),
        "sel": sel,
        "onesv": np.ones((P, 4, NPAIR, 64), np.float16),
    }


def kernel(x, W_qkv, b_qkv, W_out, b_out):
    x = np.asarray(x, np.float32)
    W_qkv = np.asarray(W_qkv, np.float32)
    b_qkv = np.asarray(b_qkv, np.float32)
    W_out = np.asarray(W_out, np.float32)
    b_out = np.asarray(b_out, np.float32)
    nc = build_nc()
    in_maps = [prep_core_inputs(x, W_qkv, b_qkv, W_out, c)
               for c in range(N_CORES)]
    res = run_bass_kernel_spmd(nc, in_maps, core_ids=list(range(N_CORES)))
    out = np.empty((B, S, D_MODEL), np.float32)
    for b in range(B):
        out[b] = (res.results[2 * b]["out"].astype(np.float32)
                  + res.results[2 * b + 1]["out"].astype(np.float32) + b_out)
    return out

